# revision 37
# baseline (speedup 1.0000x reference)
"""Trainium2 Bass kernel for multi-head attention (GQA + RoPE + causal).

Problem shapes (hardcoded): x [2, 2048, 2048] f32, w_qkv [3072, 2048],
w_o [2048, 2048], position_ids [2, 2048] int, mask [1,1,2048,2048] causal.

Sharding: 8 cores = 2 batches x 4 KV-head groups. Each core computes, for
one batch b and one kv-group g (4 query heads + 1 kv head):
  - Y^T = (w_shard @ x[b]^T) in "feature-major" layout [f, s] (bf16 matmuls),
    processed s-slice-major so projection, attention and o_proj pipeline
  - RoPE on Q^T/K^T (tables precomputed on host from position_ids)
  - causal attention in transposed-score layout S_T[k, q] (no transposes;
    no max subtraction needed at these score magnitudes |s| < ~10).
    softmax denominators are accumulated on DVE (running add of exp tiles)
    + one gpsimd partition_all_reduce per (q-tile, head) — keeps the PE free
    of the ones-vector sum matmuls
  - partial o_proj out^T[oc, s] = w_o_slice^T @ A^T, stored bf16
Host sums the 4 bf16 partials per batch and transposes back.
"""

import math
from contextlib import ExitStack
from dataclasses import dataclass

import numpy as np
import ml_dtypes

import concourse.bass as bass
import concourse.tile as tile
from concourse import bacc, bass_isa, mybir
from concourse.masks import make_identity

P = 128
BF16 = mybir.dt.bfloat16
F32 = mybir.dt.float32
BF16_NP = ml_dtypes.bfloat16

# full-size problem constants
B, S_FULL, HID_FULL = 2, 2048, 2048
NH, NKV, HD = 16, 4, 128
NQL_HD = (NH // NKV) * HD  # 512
ROPE_BASE = 10000.0
N_CORES = 8


@dataclass(frozen=True)
class Cfg:
    S: int = S_FULL          # sequence length
    HID: int = HID_FULL      # model dim (contraction for qkv proj)
    NQL: int = NH // NKV     # local query heads per core
    QT: int = 512            # q tile (matmul free dim)

    @property
    def HT(self):            # contraction tiles for qkv proj
        return self.HID // P

    @property
    def NS(self):            # s-slices of size QT
        return self.S // self.QT

    @property
    def NQT(self):           # q tiles per head
        return self.S // self.QT

    @property
    def NKT(self):           # k tiles (128 wide)
        return self.S // P

    @property
    def FQK(self):           # 128-blocks of qk features (NQL q heads + 1 k head)
        return self.NQL + 1

    @property
    def OC(self):            # o_proj output features (full hidden)
        return self.HID

    @property
    def TPQ(self):           # k tiles per q tile (causal step)
        return self.QT // P


def emit(ctx: ExitStack, tc: tile.TileContext, cfg: Cfg, io: dict, n_reps: int = 1):
    res = ctx.enter_context(tc.tile_pool(name="res", bufs=1))
    work = ctx.enter_context(tc.tile_pool(name="work", bufs=1))
    ps = ctx.enter_context(tc.tile_pool(name="ps", bufs=1, space="PSUM"))
    for rep in range(n_reps):  # >1 only for timing builds
        # accumulate into outT on reps > 0 so repeats aren't dead-code
        # eliminated by the NEFF compiler (timing builds only)
        emit_once(tc, cfg, io, res, work, ps, accum=(rep > 0))


def emit_once(tc: tile.TileContext, cfg: Cfg, io: dict, res, work, ps, accum=False):
    nc = tc.nc
    S, QT, HT, NQL, NS = cfg.S, cfg.QT, cfg.HT, cfg.NQL, cfg.NS
    xS, wqkT, wvT, woT, cosT, sinT, outT = (
        io["xS"], io["wqkT"], io["wvT"], io["woT"], io["cosT"], io["sinT"],
        io["outT"],
    )
    tri = io["tri"]
    trineg = io["trineg"]

    # ---- resident tiles ----
    wqk_sb = res.tile([P, HT, cfg.FQK * P], BF16, tag="wqk", name="wqk_sb")
    wqk_r = wqkT.rearrange("(ht p) f -> p ht f", p=P)
    wv_sb = res.tile([P, HT, P], BF16, tag="wv", name="wv_sb")
    wv_r = wvT.rearrange("(ht p) f -> p ht f", p=P)
    wo_sb = res.tile([P, NQL, cfg.OC], BF16, tag="wo", name="wo_sb")
    cos_sb = res.tile([P, S], BF16, tag="cos", name="cos_sb")
    sin_sb = res.tile([P, S], BF16, tag="sin", name="sin_sb")
    tri_sb = res.tile([P, cfg.TPQ, QT], BF16, tag="tri", name="tri_sb")
    trineg_sb = res.tile([P, P], BF16, tag="trineg", name="trineg_sb")
    ident_sb = res.tile([P, P], BF16, tag="ident", name="ident_sb")

    kT_sb = res.tile([P, S], BF16, tag="kT", name="kT_sb")   # roped K^T
    qT_sb = res.tile([P, NQL, S], BF16, tag="qT", name="qT_sb")  # roped, scaled
    v_sb = res.tile([P, cfg.NKT, P], BF16, tag="v", name="v_sb")  # V natural
    a_sb = res.tile([P, NQL, S], BF16, tag="a", name="a_sb")  # attention out

    # x s-slices, chunked DMA so first matmuls start early
    NCH = 4
    CH = HT // NCH
    xS_r = xS.rearrange("n (ht p) q -> p n ht q", p=P)

    kf = bass.ts(NQL, P)        # K feature columns of wqk
    qf = slice(0, NQL * P)      # Q feature columns

    # ---- DMA schedule: what the first slices need comes first; weight
    # and x loads chunked by h so the first accumulation chains can start
    # after the first chunk lands ----
    x_slices = []
    for si in range(NS):
        xs = work.tile([P, HT, QT], BF16, tag="xs", bufs=3, name=f"xs{si}")
        x_slices.append(xs)

    def dma_x_slice(si):
        for c in range(NCH):
            hs = slice(c * CH, (c + 1) * CH)
            nc.sync.dma_start(out=x_slices[si][:, hs, :], in_=xS_r[:, si, hs, :])

    for c in range(NCH):
        hs = slice(c * CH, (c + 1) * CH)
        nc.sync.dma_start(out=wqk_sb[:, hs, kf], in_=wqk_r[:, hs, kf])
        nc.sync.dma_start(out=x_slices[0][:, hs, :], in_=xS_r[:, 0, hs, :])
    nc.sync.dma_start(out=cos_sb[:], in_=cosT[:, :])
    nc.sync.dma_start(out=sin_sb[:], in_=sinT[:, :])
    for c in range(NCH):
        hs = slice(c * CH, (c + 1) * CH)
        nc.sync.dma_start(out=wv_sb[:, hs, :], in_=wv_r[:, hs, :])
        nc.sync.dma_start(out=wqk_sb[:, hs, qf], in_=wqk_r[:, hs, qf])
    dma_x_slice(1)
    nc.sync.dma_start(out=tri_sb[:], in_=tri.rearrange("(d p) q -> p d q", p=P))
    nc.sync.dma_start(out=trineg_sb[:], in_=trineg[:, :])
    make_identity(nc, ident_sb[:])
    dma_x_slice(2)
    nc.sync.dma_start(out=wo_sb[:], in_=woT.rearrange("(fq p) oc -> p fq oc", p=P))
    dma_x_slice(3)

    outT_r = outT.rearrange("(g i p) s -> p g i s", p=P, i=4)

    # ---- projection helper (Y^T for one 128-wide feature block) ----
    def proj_block(fslice, si, dst, do_rope, w_sb=None):
        sl = bass.ts(si, QT)
        wsb = wqk_sb if w_sb is None else w_sb
        acc = ps.tile([P, QT], F32, tag="mm", bufs=3, name="acc")
        for hi in range(HT):
            nc.tensor.matmul(
                acc[:], wsb[:, hi, fslice], x_slices[si][:, hi, :],
                start=(hi == 0), stop=(hi == HT - 1),
            )
        y = work.tile([P, QT], BF16, tag="y", bufs=6, name="y")
        nc.vector.tensor_copy(y[:], acc[:])
        if not do_rope:
            return y
        # rope: out = y*cos + swap_halves(y)*sin'
        # (sin' is pre-negated in its lower half on host).
        # Half-swap via 1-input copies: 2-input DVE ops require equal
        # SBUF base partitions on HW.
        sw = work.tile([P, QT], BF16, tag="sw", bufs=4, name="sw")
        nc.vector.tensor_copy(sw[0:64, :], y[64:128, :])
        nc.vector.tensor_copy(sw[64:128, :], y[0:64, :])
        t1 = work.tile([P, QT], BF16, tag="t1", bufs=4, name="t1")
        nc.vector.tensor_mul(t1[:], sw[:], sin_sb[:, sl])
        t2 = work.tile([P, QT], BF16, tag="t2", bufs=4, name="t2")
        nc.vector.tensor_mul(t2[:], y[:], cos_sb[:, sl])
        nc.vector.tensor_add(dst, t2[:], t1[:])
        return None

    def emit_proj_slice(si):
        with nc.named_scope(f"proj_s{si}"):
            # K first (attention needs it before q heads), then V, then Q
            proj_block(bass.ts(NQL, P), si, kT_sb[:, bass.ts(si, QT)], True)
            vt = proj_block(slice(0, P), si, None, False, w_sb=wv_sb)
            for j in range(QT // P):
                pst = ps.tile([P, P], BF16, tag="mm", bufs=3, name="pst")
                nc.tensor.transpose(pst[:], vt[:, bass.ts(j, P)], ident_sb[:])
                nc.vector.tensor_copy(v_sb[:, si * (QT // P) + j, :], pst[:])
            for fi in range(NQL):
                proj_block(bass.ts(fi, P), si,
                           qT_sb[:, fi, bass.ts(si, QT)], True)

    def emit_attn_tile(t):
        qsl = bass.ts(t, QT)
        nk = (t + 1) * cfg.TPQ  # valid k tiles (causal)
        with nc.named_scope(f"attn_t{t}"):
            for h in range(NQL):
                pv_ps = ps.tile([P, QT], F32, tag="pv", bufs=2, name="pv_ps")
                run = work.tile([P, QT], BF16, tag="run", bufs=3, name="run")
                for j in range(nk):
                    # boundary tiles (d > 0): columns q' < d fully masked;
                    # restrict the whole chain to [d:QT].
                    d = max(0, j * P - t * QT)
                    s_ps = ps.tile([P, QT], F32, tag="s", bufs=3, name="s_ps")
                    p_sb = work.tile([P, QT], BF16, tag="p", bufs=8, name="p_sb")
                    nc.tensor.matmul(
                        s_ps[:, d:QT],
                        kT_sb[:, bass.ts(j, P)],
                        qT_sb[:, h, t * QT + d:(t + 1) * QT],
                        start=True, stop=True,
                    )
                    if j * P - t * QT >= 0:
                        # diagonal 128-block: add -30 outside the causal
                        # triangle BEFORE exp, so the mask gates the (slower)
                        # exp instead of sitting between exp and PV
                        nc.vector.tensor_add(
                            s_ps[:, d:d + P], s_ps[:, d:d + P],
                            trineg_sb[:, 0:P],
                        )
                    nc.scalar.activation(
                        p_sb[:, d:QT], s_ps[:, d:QT],
                        mybir.ActivationFunctionType.Exp,
                    )
                    # softmax denominator: running add on DVE (bf16 2x)
                    if j == 0:
                        nc.vector.tensor_copy(run[:], p_sb[:])
                    else:
                        nc.vector.tensor_add(run[:, d:QT], run[:, d:QT],
                                             p_sb[:, d:QT])
                    nc.tensor.matmul(
                        pv_ps[:, d:QT], v_sb[:, j, :], p_sb[:, d:QT],
                        start=(j == 0), stop=(j == nk - 1),
                    )
                # cross-partition reduce of run -> every partition has sums
                bc = work.tile([P, QT], F32, tag="bc", bufs=2, name="bc")
                nc.gpsimd.partition_all_reduce(
                    bc[:], run[:], channels=P, reduce_op=bass_isa.ReduceOp.add,
                )
                rec = work.tile([P, QT], F32, tag="rec", bufs=2, name="rec")
                nc.vector.reciprocal(rec[:], bc[:])
                nc.vector.tensor_mul(a_sb[:, h, qsl], pv_ps[:], rec[:])

    def emit_oproj_tile(t):
        qsl = bass.ts(t, QT)
        with nc.named_scope(f"oproj_t{t}"):
            for g in range(cfg.OC // P // 4):  # groups of 4 output blocks
                orow = work.tile([P, 4, QT], BF16, tag="orow", bufs=3,
                                 name="orow")
                for i in range(4):
                    oi = g * 4 + i
                    acc = ps.tile([P, QT], F32, tag="mm", bufs=3, name="acc_o")
                    for fi in range(NQL):
                        nc.tensor.matmul(
                            acc[:], wo_sb[:, fi, bass.ts(oi, P)],
                            a_sb[:, fi, qsl],
                            start=(fi == 0), stop=(fi == NQL - 1),
                        )
                    if accum and oi == 0 and t == 0:
                        # timing builds: chain on previous rep's output so the
                        # NEFF compiler can't dead-code-eliminate earlier reps
                        prev = work.tile([P, QT], BF16, tag="prev", bufs=1,
                                         name="prev")
                        nc.sync.dma_start(out=prev[:], in_=outT[0:P, 0:QT])
                        nc.vector.tensor_add(orow[:, i, :], acc[:], prev[:])
                    elif (g * 4 + i) % 2 == 0:
                        nc.scalar.copy(orow[:, i, :], acc[:])
                    else:
                        nc.vector.tensor_copy(orow[:, i, :], acc[:])
                nc.sync.dma_start(out=outT_r[:, g, :, qsl], in_=orow[:])

    # ---- interleaved emission: o_proj(t) emitted one tile late so the
    # next tile's attention chain outranks it in scheduler priority ----
    emit_proj_slice(0)
    for t in range(cfg.NQT):
        if t + 1 < NS:
            emit_proj_slice(t + 1)
        emit_attn_tile(t)
        if t > 0:
            emit_oproj_tile(t - 1)
    emit_oproj_tile(cfg.NQT - 1)


def build(cfg: Cfg, n_reps: int = 1):
    nc = bacc.Bacc("TRN2", target_bir_lowering=False, debug=False)
    io = {
        "xS": nc.dram_tensor("xS", [cfg.NS, cfg.HID, cfg.QT], BF16, kind="ExternalInput").ap(),
        "wqkT": nc.dram_tensor("wqkT", [cfg.HID, cfg.FQK * P], BF16, kind="ExternalInput").ap(),
        "wvT": nc.dram_tensor("wvT", [cfg.HID, P], BF16, kind="ExternalInput").ap(),
        "woT": nc.dram_tensor("woT", [cfg.NQL * P, cfg.OC], BF16, kind="ExternalInput").ap(),
        "cosT": nc.dram_tensor("cosT", [P, cfg.S], BF16, kind="ExternalInput").ap(),
        "sinT": nc.dram_tensor("sinT", [P, cfg.S], BF16, kind="ExternalInput").ap(),
        "tri": nc.dram_tensor("tri", [(cfg.QT // P) * P, cfg.QT], BF16, kind="ExternalInput").ap(),
        "trineg": nc.dram_tensor("trineg", [P, P], BF16, kind="ExternalInput").ap(),
        "outT": nc.dram_tensor("outT", [cfg.OC, cfg.S], BF16, kind="ExternalOutput").ap(),
    }
    with tile.TileContext(nc) as tc:
        with ExitStack() as ctx:
            emit(ctx, tc, cfg, io, n_reps=n_reps)
    nc.compile()
    return nc


def rope_tables(position_ids_b: np.ndarray):
    """cos/sin tables in [d, s] layout, both halves stacked; sin lower half
    negated (so rope = y*cos + swap(y)*sin)."""
    half = HD // 2
    inv_freq = 1.0 / (ROPE_BASE ** (np.arange(half, dtype=np.float64) / half))
    freqs = np.asarray(position_ids_b, dtype=np.float64)[None, :] * inv_freq[:, None]
    cos = np.cos(freqs)
    sin = np.sin(freqs)
    cosT = np.concatenate([cos, cos], 0)
    sinT = np.concatenate([-sin, sin], 0)
    return cosT, sinT


def make_in_maps(x, position_ids, w_qkv, w_o):
    """Shard full inputs into per-core input maps (host-side prep)."""
    q_dim = NH * HD
    kv_dim = NKV * HD
    in_maps = []
    tri = make_tri(512)
    trineg = make_trineg()
    scale = 1.0 / math.sqrt(HD)
    tabs = {}
    xs = {}
    for b in range(B):
        cosT, sinT = rope_tables(position_ids[b])
        tabs[b] = (cosT.astype(BF16_NP), sinT.astype(BF16_NP))
        # s-major slices: [NS, HID, QT]
        xT = np.ascontiguousarray(x[b].T).astype(BF16_NP)
        xs[b] = np.ascontiguousarray(
            xT.reshape(HID_FULL, S_FULL // 512, 512).transpose(1, 0, 2))
    for c in range(N_CORES):
        b, g = divmod(c, NKV)
        # weights for this core's heads: 4 q heads (pre-scaled), 1 k, 1 v head
        wq = w_qkv[g * NQL_HD:(g + 1) * NQL_HD, :] * scale
        wk = w_qkv[q_dim + g * HD:q_dim + (g + 1) * HD, :]
        wv = w_qkv[q_dim + kv_dim + g * HD:q_dim + kv_dim + (g + 1) * HD, :]
        wqkT = np.ascontiguousarray(np.concatenate([wq, wk], 0).T).astype(BF16_NP)
        wvT = np.ascontiguousarray(wv.T).astype(BF16_NP)
        woT = np.ascontiguousarray(w_o.T[g * NQL_HD:(g + 1) * NQL_HD, :]).astype(BF16_NP)
        in_maps.append({
            "xS": xs[b],
            "wqkT": wqkT,
            "wvT": wvT,
            "woT": woT,
            "cosT": tabs[b][0],
            "sinT": tabs[b][1],
            "tri": tri,
            "trineg": trineg,
        })
    return in_maps


def make_tri(QT):
    """Stacked boundary masks: tri[d*128+k, q] = 1 if q >= k + d*128."""
    k = np.arange(P)
    q = np.arange(QT)
    blocks = [(q[None, :] >= (k[:, None] + d)) for d in range(0, QT, P)]
    return np.concatenate(blocks, 0).astype(BF16_NP)


def make_trineg():
    """Additive causal bias for a diagonal 128-block: 0 on/below, -30 above
    (exp(-30-ish) ~ 1e-13 — negligible against sums ~ e^8)."""
    k = np.arange(P)
    q = np.arange(P)
    return np.where(q[None, :] >= k[:, None], 0.0, -30.0).astype(BF16_NP)


def _causal_mask_ok(mask):
    m = np.asarray(mask)
    if m.shape != (1, 1, S_FULL, S_FULL):
        return False
    tril = np.tril(np.ones((S_FULL, S_FULL), dtype=bool))
    m0 = m[0, 0]
    return bool((m0[tril] == 0.0).all() and (m0[~tril] <= -1e8).all())


def _reference_numpy(x, position_ids, mask, w_qkv, w_o):
    """Fallback (never expected to trigger): plain numpy reference."""
    half = HD // 2

    def rope(v, pos):
        inv_freq = 1.0 / (ROPE_BASE ** (np.arange(half) / half))
        f = np.asarray(pos, dtype=np.float64)[:, None] * inv_freq[None, :]
        cos, sin = np.cos(f), np.sin(f)
        x1, x2 = v[..., :half], v[..., half:]
        return np.concatenate([x1 * cos - x2 * sin, x2 * cos + x1 * sin], -1)

    out = np.empty((B, S_FULL, HID_FULL), np.float32)
    q_dim, kv_dim = NH * HD, NKV * HD
    xd = x.astype(np.float64)
    for b in range(B):
        qkv = xd[b] @ w_qkv.T.astype(np.float64)
        q = qkv[:, :q_dim].reshape(S_FULL, NH, HD).transpose(1, 0, 2)
        k = qkv[:, q_dim:q_dim + kv_dim].reshape(S_FULL, NKV, HD).transpose(1, 0, 2)
        v = qkv[:, q_dim + kv_dim:].reshape(S_FULL, NKV, HD).transpose(1, 0, 2)
        q = np.stack([rope(qh, position_ids[b]) for qh in q])
        k = np.stack([rope(kh, position_ids[b]) for kh in k])
        rep = NH // NKV
        acc = np.empty((S_FULL, NH, HD))
        for h in range(NH):
            s = q[h] @ k[h // rep].T / math.sqrt(HD) + mask[0, 0]
            s -= s.max(-1, keepdims=True)
            e = np.exp(s)
            p = e / e.sum(-1, keepdims=True)
            acc[:, h, :] = p @ v[h // rep]
        out[b] = (acc.reshape(S_FULL, NH * HD) @ w_o.T.astype(np.float64)).astype(np.float32)
    return out


_NC_CACHE = {}


def _get_nc():
    if "full" not in _NC_CACHE:
        _NC_CACHE["full"] = build(Cfg())
    return _NC_CACHE["full"]


def kernel(x, position_ids, mask, w_qkv, w_o):
    x = np.asarray(x, dtype=np.float32)
    position_ids = np.asarray(position_ids)
    w_qkv = np.asarray(w_qkv, dtype=np.float32)
    w_o = np.asarray(w_o, dtype=np.float32)
    if not _causal_mask_ok(mask):
        return _reference_numpy(x, position_ids, np.asarray(mask, np.float32),
                                w_qkv, w_o)

    from concourse.bass_utils import run_bass_kernel_spmd

    nc = _get_nc()
    in_maps = make_in_maps(x, position_ids, w_qkv, w_o)
    res = run_bass_kernel_spmd(nc, in_maps, list(range(N_CORES)))
    out = np.empty((B, S_FULL, HID_FULL), dtype=np.float32)
    for b in range(B):
        acc = res.results[b * NKV + 0]["outT"].astype(np.float32)
        for g in range(1, NKV):
            acc = acc + res.results[b * NKV + g]["outT"].astype(np.float32)
        out[b] = acc.T
    return out


# revision 39
# speedup vs baseline: 1.0467x; 1.0467x over previous
"""Trainium2 Bass kernel for multi-head attention (GQA + RoPE + causal).

Problem shapes (hardcoded): x [2, 2048, 2048] f32, w_qkv [3072, 2048],
w_o [2048, 2048], position_ids [2, 2048] int, mask [1,1,2048,2048] causal.

Sharding: 8 cores = 2 batches x 4 KV-head groups. Each core computes, for
one batch b and one kv-group g (4 query heads + 1 kv head):
  - Y^T = (w_shard @ x[b]^T) in "feature-major" layout [f, s] (bf16 matmuls),
    processed s-slice-major so projection, attention and o_proj pipeline
    (x is staged in DRAM as s-major slices; loads are h-chunked so the
    first accumulation chains start as soon as the first chunk lands)
  - RoPE on Q^T/K^T (tables precomputed on host from position_ids)
  - causal attention in transposed-score layout S_T[k, q] (no transposes;
    no max subtraction needed at these score magnitudes |s| < ~10).
    The causal mask for diagonal 128-blocks is applied as a -30 additive
    bias on the scores BEFORE exp (so it gates the exp, not the PE's PV
    matmul). Softmax denominators are accumulated on DVE (running bf16 add
    of exp tiles) + one gpsimd partition_all_reduce per (q-tile, head) —
    keeps the PE free of ones-vector sum matmuls.
  - partial o_proj out^T[oc, s] = w_o_slice^T @ A^T, stored bf16; its
    emission is delayed one q-tile so the next tile's latency-critical
    attention chain outranks it in scheduler priority (o_proj is fill work)
Host sums the 4 bf16 partials per batch and transposes back.
"""

import math
from contextlib import ExitStack
from dataclasses import dataclass

import numpy as np
import ml_dtypes

import concourse.bass as bass
import concourse.tile as tile
from concourse import bacc, bass_isa, mybir
from concourse.masks import make_identity

P = 128
BF16 = mybir.dt.bfloat16
F32 = mybir.dt.float32
BF16_NP = ml_dtypes.bfloat16

# full-size problem constants
B, S_FULL, HID_FULL = 2, 2048, 2048
NH, NKV, HD = 16, 4, 128
NQL_HD = (NH // NKV) * HD  # 512
ROPE_BASE = 10000.0
N_CORES = 8


@dataclass(frozen=True)
class Cfg:
    S: int = S_FULL          # sequence length
    HID: int = HID_FULL      # model dim (contraction for qkv proj)
    NQL: int = NH // NKV     # local query heads per core
    QT: int = 512            # q tile (matmul free dim)

    @property
    def HT(self):            # contraction tiles for qkv proj
        return self.HID // P

    @property
    def NS(self):            # s-slices of size QT
        return self.S // self.QT

    @property
    def NQT(self):           # q tiles per head
        return self.S // self.QT

    @property
    def NKT(self):           # k tiles (128 wide)
        return self.S // P

    @property
    def FQK(self):           # 128-blocks of qk features (NQL q heads + 1 k head)
        return self.NQL + 1

    @property
    def OC(self):            # o_proj output features (full hidden)
        return self.HID

    @property
    def TPQ(self):           # k tiles per q tile (causal step)
        return self.QT // P


def emit(ctx: ExitStack, tc: tile.TileContext, cfg: Cfg, io: dict, n_reps: int = 1):
    res = ctx.enter_context(tc.tile_pool(name="res", bufs=1))
    work = ctx.enter_context(tc.tile_pool(name="work", bufs=1))
    ps = ctx.enter_context(tc.tile_pool(name="ps", bufs=1, space="PSUM"))
    for rep in range(n_reps):  # >1 only for timing builds
        # accumulate into outT on reps > 0 so repeats aren't dead-code
        # eliminated by the NEFF compiler (timing builds only)
        emit_once(tc, cfg, io, res, work, ps, accum=(rep > 0))


def emit_once(tc: tile.TileContext, cfg: Cfg, io: dict, res, work, ps, accum=False):
    nc = tc.nc
    S, QT, HT, NQL, NS = cfg.S, cfg.QT, cfg.HT, cfg.NQL, cfg.NS
    xS, wqkT, wvT, woT, cosT, sinT, outT = (
        io["xS"], io["wqkT"], io["wvT"], io["woT"], io["cosT"], io["sinT"],
        io["outT"],
    )
    trineg = io["trineg"]

    # ---- resident tiles ----
    wqk_sb = res.tile([P, HT, cfg.FQK * P], BF16, tag="wqk", name="wqk_sb")
    wqk_r = wqkT.rearrange("(ht p) f -> p ht f", p=P)
    wv_sb = res.tile([P, HT, P], BF16, tag="wv", name="wv_sb")
    wv_r = wvT.rearrange("(ht p) f -> p ht f", p=P)
    wo_sb = res.tile([P, NQL, cfg.OC], BF16, tag="wo", name="wo_sb")
    cos_sb = res.tile([P, S], BF16, tag="cos", name="cos_sb")
    sin_sb = res.tile([P, S], BF16, tag="sin", name="sin_sb")
    trineg_sb = res.tile([P, P], BF16, tag="trineg", name="trineg_sb")
    ident_sb = res.tile([P, P], BF16, tag="ident", name="ident_sb")

    kT_sb = res.tile([P, S], BF16, tag="kT", name="kT_sb")   # roped K^T
    qT_sb = res.tile([P, NQL, S], BF16, tag="qT", name="qT_sb")  # roped, scaled
    v_sb = res.tile([P, cfg.NKT, P], BF16, tag="v", name="v_sb")  # V natural
    a_sb = res.tile([P, NQL, S], BF16, tag="a", name="a_sb")  # attention out

    # x s-slices, chunked DMA so first matmuls start early
    NCH = 4
    CH = HT // NCH
    xS_r = xS.rearrange("n (ht p) q -> p n ht q", p=P)

    kf = bass.ts(NQL, P)        # K feature columns of wqk
    qf = slice(0, NQL * P)      # Q feature columns

    # ---- DMA schedule: what the first slices need comes first; weight
    # and x loads chunked by h so the first accumulation chains can start
    # after the first chunk lands ----
    x_slices = []
    for si in range(NS):
        xs = work.tile([P, HT, QT], BF16, tag="xs", bufs=3, name=f"xs{si}")
        x_slices.append(xs)

    def dma_x_slice(si):
        for c in range(NCH):
            hs = slice(c * CH, (c + 1) * CH)
            nc.sync.dma_start(out=x_slices[si][:, hs, :], in_=xS_r[:, si, hs, :])

    for c in range(NCH):
        hs = slice(c * CH, (c + 1) * CH)
        nc.sync.dma_start(out=wqk_sb[:, hs, kf], in_=wqk_r[:, hs, kf])
        nc.sync.dma_start(out=x_slices[0][:, hs, :], in_=xS_r[:, 0, hs, :])
    nc.sync.dma_start(out=cos_sb[:], in_=cosT[:, :])
    nc.sync.dma_start(out=sin_sb[:], in_=sinT[:, :])
    for c in range(NCH):
        hs = slice(c * CH, (c + 1) * CH)
        nc.sync.dma_start(out=wv_sb[:, hs, :], in_=wv_r[:, hs, :])
        nc.sync.dma_start(out=wqk_sb[:, hs, qf], in_=wqk_r[:, hs, qf])
    dma_x_slice(1)
    nc.sync.dma_start(out=trineg_sb[:], in_=trineg[:, :])
    make_identity(nc, ident_sb[:])
    dma_x_slice(2)
    nc.sync.dma_start(out=wo_sb[:], in_=woT.rearrange("(fq p) oc -> p fq oc", p=P))
    dma_x_slice(3)

    outT_r = outT.rearrange("(g i p) s -> p g i s", p=P, i=4)

    # ---- projection helper (Y^T for one 128-wide feature block) ----
    def proj_block(fslice, si, dst, do_rope, w_sb=None):
        sl = bass.ts(si, QT)
        wsb = wqk_sb if w_sb is None else w_sb
        acc = ps.tile([P, QT], F32, tag="mm", bufs=3, name="acc")
        for hi in range(HT):
            nc.tensor.matmul(
                acc[:], wsb[:, hi, fslice], x_slices[si][:, hi, :],
                start=(hi == 0), stop=(hi == HT - 1),
            )
        y = work.tile([P, QT], BF16, tag="y", bufs=6, name="y")
        nc.vector.tensor_copy(y[:], acc[:])
        if not do_rope:
            return y
        # rope: out = y*cos + swap_halves(y)*sin'
        # (sin' is pre-negated in its lower half on host).
        # Half-swap via 1-input copies: 2-input DVE ops require equal
        # SBUF base partitions on HW.
        sw = work.tile([P, QT], BF16, tag="sw", bufs=4, name="sw")
        nc.vector.tensor_copy(sw[0:64, :], y[64:128, :])
        nc.vector.tensor_copy(sw[64:128, :], y[0:64, :])
        t1 = work.tile([P, QT], BF16, tag="t1", bufs=4, name="t1")
        nc.vector.tensor_mul(t1[:], sw[:], sin_sb[:, sl])
        t2 = work.tile([P, QT], BF16, tag="t2", bufs=4, name="t2")
        nc.vector.tensor_mul(t2[:], y[:], cos_sb[:, sl])
        nc.vector.tensor_add(dst, t2[:], t1[:])
        return None

    def emit_proj_slice(si):
        with nc.named_scope(f"proj_s{si}"):
            # K first (attention needs it before q heads), then V, then Q
            proj_block(bass.ts(NQL, P), si, kT_sb[:, bass.ts(si, QT)], True)
            vt = proj_block(slice(0, P), si, None, False, w_sb=wv_sb)
            for j in range(QT // P):
                pst = ps.tile([P, P], BF16, tag="mm", bufs=3, name="pst")
                nc.tensor.transpose(pst[:], vt[:, bass.ts(j, P)], ident_sb[:])
                nc.vector.tensor_copy(v_sb[:, si * (QT // P) + j, :], pst[:])
            for fi in range(NQL):
                proj_block(bass.ts(fi, P), si,
                           qT_sb[:, fi, bass.ts(si, QT)], True)

    def emit_attn_tile(t):
        qsl = bass.ts(t, QT)
        nk = (t + 1) * cfg.TPQ  # valid k tiles (causal)
        with nc.named_scope(f"attn_t{t}"):
            for h in range(NQL):
                pv_ps = ps.tile([P, QT], F32, tag="pv", bufs=2, name="pv_ps")
                run = work.tile([P, QT], BF16, tag="run", bufs=3, name="run")
                for j in range(nk):
                    # boundary tiles (d > 0): columns q' < d fully masked;
                    # restrict the whole chain to [d:QT].
                    d = max(0, j * P - t * QT)
                    s_ps = ps.tile([P, QT], F32, tag="s", bufs=3, name="s_ps")
                    p_sb = work.tile([P, QT], BF16, tag="p", bufs=8, name="p_sb")
                    nc.tensor.matmul(
                        s_ps[:, d:QT],
                        kT_sb[:, bass.ts(j, P)],
                        qT_sb[:, h, t * QT + d:(t + 1) * QT],
                        start=True, stop=True,
                    )
                    if j * P - t * QT >= 0:
                        # diagonal 128-block: add -30 outside the causal
                        # triangle BEFORE exp, so the mask gates the (slower)
                        # exp instead of sitting between exp and PV
                        nc.vector.tensor_add(
                            s_ps[:, d:d + P], s_ps[:, d:d + P],
                            trineg_sb[:, 0:P],
                        )
                    nc.scalar.activation(
                        p_sb[:, d:QT], s_ps[:, d:QT],
                        mybir.ActivationFunctionType.Exp,
                    )
                    # softmax denominator: running add on DVE (bf16 2x)
                    if j == 0:
                        nc.vector.tensor_copy(run[:], p_sb[:])
                    else:
                        nc.vector.tensor_add(run[:, d:QT], run[:, d:QT],
                                             p_sb[:, d:QT])
                    nc.tensor.matmul(
                        pv_ps[:, d:QT], v_sb[:, j, :], p_sb[:, d:QT],
                        start=(j == 0), stop=(j == nk - 1),
                    )
                # cross-partition reduce of run -> every partition has sums
                bc = work.tile([P, QT], F32, tag="bc", bufs=2, name="bc")
                nc.gpsimd.partition_all_reduce(
                    bc[:], run[:], channels=P, reduce_op=bass_isa.ReduceOp.add,
                )
                rec = work.tile([P, QT], F32, tag="rec", bufs=2, name="rec")
                nc.vector.reciprocal(rec[:], bc[:])
                nc.vector.tensor_mul(a_sb[:, h, qsl], pv_ps[:], rec[:])

    def emit_oproj_tile(t):
        qsl = bass.ts(t, QT)
        with nc.named_scope(f"oproj_t{t}"):
            for g in range(cfg.OC // P // 4):  # groups of 4 output blocks
                orow = work.tile([P, 4, QT], BF16, tag="orow", bufs=3,
                                 name="orow")
                for i in range(4):
                    oi = g * 4 + i
                    acc = ps.tile([P, QT], F32, tag="mm", bufs=3, name="acc_o")
                    for fi in range(NQL):
                        nc.tensor.matmul(
                            acc[:], wo_sb[:, fi, bass.ts(oi, P)],
                            a_sb[:, fi, qsl],
                            start=(fi == 0), stop=(fi == NQL - 1),
                        )
                    if accum and oi == 0 and t == 0:
                        # timing builds: chain on previous rep's output so the
                        # NEFF compiler can't dead-code-eliminate earlier reps
                        prev = work.tile([P, QT], BF16, tag="prev", bufs=1,
                                         name="prev")
                        nc.sync.dma_start(out=prev[:], in_=outT[0:P, 0:QT])
                        nc.vector.tensor_add(orow[:, i, :], acc[:], prev[:])
                    elif (g * 4 + i) % 2 == 0:
                        nc.scalar.copy(orow[:, i, :], acc[:])
                    else:
                        nc.vector.tensor_copy(orow[:, i, :], acc[:])
                nc.sync.dma_start(out=outT_r[:, g, :, qsl], in_=orow[:])

    # ---- interleaved emission: o_proj(t) emitted one tile late so the
    # next tile's attention chain outranks it in scheduler priority ----
    emit_proj_slice(0)
    for t in range(cfg.NQT):
        if t + 1 < NS:
            emit_proj_slice(t + 1)
        emit_attn_tile(t)
        if t > 0:
            emit_oproj_tile(t - 1)
    emit_oproj_tile(cfg.NQT - 1)


def build(cfg: Cfg, n_reps: int = 1):
    nc = bacc.Bacc("TRN2", target_bir_lowering=False, debug=False)
    io = {
        "xS": nc.dram_tensor("xS", [cfg.NS, cfg.HID, cfg.QT], BF16, kind="ExternalInput").ap(),
        "wqkT": nc.dram_tensor("wqkT", [cfg.HID, cfg.FQK * P], BF16, kind="ExternalInput").ap(),
        "wvT": nc.dram_tensor("wvT", [cfg.HID, P], BF16, kind="ExternalInput").ap(),
        "woT": nc.dram_tensor("woT", [cfg.NQL * P, cfg.OC], BF16, kind="ExternalInput").ap(),
        "cosT": nc.dram_tensor("cosT", [P, cfg.S], BF16, kind="ExternalInput").ap(),
        "sinT": nc.dram_tensor("sinT", [P, cfg.S], BF16, kind="ExternalInput").ap(),
        "trineg": nc.dram_tensor("trineg", [P, P], BF16, kind="ExternalInput").ap(),
        "outT": nc.dram_tensor("outT", [cfg.OC, cfg.S], BF16, kind="ExternalOutput").ap(),
    }
    with tile.TileContext(nc) as tc:
        with ExitStack() as ctx:
            emit(ctx, tc, cfg, io, n_reps=n_reps)
    nc.compile()
    return nc


def rope_tables(position_ids_b: np.ndarray):
    """cos/sin tables in [d, s] layout, both halves stacked; sin lower half
    negated (so rope = y*cos + swap(y)*sin)."""
    half = HD // 2
    inv_freq = 1.0 / (ROPE_BASE ** (np.arange(half, dtype=np.float64) / half))
    freqs = np.asarray(position_ids_b, dtype=np.float64)[None, :] * inv_freq[:, None]
    cos = np.cos(freqs)
    sin = np.sin(freqs)
    cosT = np.concatenate([cos, cos], 0)
    sinT = np.concatenate([-sin, sin], 0)
    return cosT, sinT


def make_in_maps(x, position_ids, w_qkv, w_o):
    """Shard full inputs into per-core input maps (host-side prep)."""
    q_dim = NH * HD
    kv_dim = NKV * HD
    in_maps = []
    trineg = make_trineg()
    scale = 1.0 / math.sqrt(HD)
    tabs = {}
    xs = {}
    for b in range(B):
        cosT, sinT = rope_tables(position_ids[b])
        tabs[b] = (cosT.astype(BF16_NP), sinT.astype(BF16_NP))
        # s-major slices: [NS, HID, QT]
        xT = np.ascontiguousarray(x[b].T).astype(BF16_NP)
        xs[b] = np.ascontiguousarray(
            xT.reshape(HID_FULL, S_FULL // 512, 512).transpose(1, 0, 2))
    for c in range(N_CORES):
        b, g = divmod(c, NKV)
        # weights for this core's heads: 4 q heads (pre-scaled), 1 k, 1 v head
        wq = w_qkv[g * NQL_HD:(g + 1) * NQL_HD, :] * scale
        wk = w_qkv[q_dim + g * HD:q_dim + (g + 1) * HD, :]
        wv = w_qkv[q_dim + kv_dim + g * HD:q_dim + kv_dim + (g + 1) * HD, :]
        wqkT = np.ascontiguousarray(np.concatenate([wq, wk], 0).T).astype(BF16_NP)
        wvT = np.ascontiguousarray(wv.T).astype(BF16_NP)
        woT = np.ascontiguousarray(w_o.T[g * NQL_HD:(g + 1) * NQL_HD, :]).astype(BF16_NP)
        in_maps.append({
            "xS": xs[b],
            "wqkT": wqkT,
            "wvT": wvT,
            "woT": woT,
            "cosT": tabs[b][0],
            "sinT": tabs[b][1],
            "trineg": trineg,
        })
    return in_maps


def make_trineg():
    """Additive causal bias for a diagonal 128-block: 0 on/below, -30 above
    (exp(-30-ish) ~ 1e-13 — negligible against sums ~ e^8)."""
    k = np.arange(P)
    q = np.arange(P)
    return np.where(q[None, :] >= k[:, None], 0.0, -30.0).astype(BF16_NP)


def _causal_mask_ok(mask):
    m = np.asarray(mask)
    if m.shape != (1, 1, S_FULL, S_FULL):
        return False
    tril = np.tril(np.ones((S_FULL, S_FULL), dtype=bool))
    m0 = m[0, 0]
    return bool((m0[tril] == 0.0).all() and (m0[~tril] <= -1e8).all())


def _reference_numpy(x, position_ids, mask, w_qkv, w_o):
    """Fallback (never expected to trigger): plain numpy reference."""
    half = HD // 2

    def rope(v, pos):
        inv_freq = 1.0 / (ROPE_BASE ** (np.arange(half) / half))
        f = np.asarray(pos, dtype=np.float64)[:, None] * inv_freq[None, :]
        cos, sin = np.cos(f), np.sin(f)
        x1, x2 = v[..., :half], v[..., half:]
        return np.concatenate([x1 * cos - x2 * sin, x2 * cos + x1 * sin], -1)

    out = np.empty((B, S_FULL, HID_FULL), np.float32)
    q_dim, kv_dim = NH * HD, NKV * HD
    xd = x.astype(np.float64)
    for b in range(B):
        qkv = xd[b] @ w_qkv.T.astype(np.float64)
        q = qkv[:, :q_dim].reshape(S_FULL, NH, HD).transpose(1, 0, 2)
        k = qkv[:, q_dim:q_dim + kv_dim].reshape(S_FULL, NKV, HD).transpose(1, 0, 2)
        v = qkv[:, q_dim + kv_dim:].reshape(S_FULL, NKV, HD).transpose(1, 0, 2)
        q = np.stack([rope(qh, position_ids[b]) for qh in q])
        k = np.stack([rope(kh, position_ids[b]) for kh in k])
        rep = NH // NKV
        acc = np.empty((S_FULL, NH, HD))
        for h in range(NH):
            s = q[h] @ k[h // rep].T / math.sqrt(HD) + mask[0, 0]
            s -= s.max(-1, keepdims=True)
            e = np.exp(s)
            p = e / e.sum(-1, keepdims=True)
            acc[:, h, :] = p @ v[h // rep]
        out[b] = (acc.reshape(S_FULL, NH * HD) @ w_o.T.astype(np.float64)).astype(np.float32)
    return out


_NC_CACHE = {}


def _get_nc():
    if "full" not in _NC_CACHE:
        _NC_CACHE["full"] = build(Cfg())
    return _NC_CACHE["full"]


def kernel(x, position_ids, mask, w_qkv, w_o):
    x = np.asarray(x, dtype=np.float32)
    position_ids = np.asarray(position_ids)
    w_qkv = np.asarray(w_qkv, dtype=np.float32)
    w_o = np.asarray(w_o, dtype=np.float32)
    if not _causal_mask_ok(mask):
        return _reference_numpy(x, position_ids, np.asarray(mask, np.float32),
                                w_qkv, w_o)

    from concourse.bass_utils import run_bass_kernel_spmd

    nc = _get_nc()
    in_maps = make_in_maps(x, position_ids, w_qkv, w_o)
    res = run_bass_kernel_spmd(nc, in_maps, list(range(N_CORES)))
    out = np.empty((B, S_FULL, HID_FULL), dtype=np.float32)
    for b in range(B):
        acc = res.results[b * NKV + 0]["outT"].astype(np.float32)
        for g in range(1, NKV):
            acc = acc + res.results[b * NKV + g]["outT"].astype(np.float32)
        out[b] = acc.T
    return out
